# revision 1
# baseline (speedup 1.0000x reference)
"""Masked 3-layer MLP (tanh) on 8 Trainium2 NeuronCores.

Reference computation (B=2048, dims 4096->8192->8192->4096, fp32):
    h1 = tanh(x @ (W1*m1).T + b1)
    h2 = tanh(h1 @ (W2*m2).T + b2)
    out =      h2 @ (W3*m3).T + b3

Strategy: Megatron-style column parallelism on every layer. Core k owns a
1/8 shard of each layer's output features (rows of W). All compute is done
in transposed orientation [features, batch] so that:
  - output features land on PSUM partitions -> per-partition bias + tanh
    fuse into the ScalarE PSUM eviction,
  - each layer's output is exactly the next layer's contraction layout,
    so no transposes are needed anywhere on device.
After layers 1 and 2 an on-chip AllGather concatenates the 8 feature shards
(concatenation is on the leading axis = features). The final layer's shard
outputs are gathered and concatenated on the host.

The mask multiply (W * m) runs on VectorE once per weight element while the
weight panel is DMA'd into SBUF; matmuls run at full rate from the cached
panel.
"""

import os
import sys

import numpy as np

for _p in ("/opt/trn_rl_repo", os.path.expanduser("~/.axon_site/_ro/trn_rl_repo")):
    if os.path.isdir(_p) and _p not in sys.path:
        sys.path.append(_p)

B = 2048
DIMS = [4096, 8192, 8192, 4096]
NCORES = 8
P = 128
FD = 512           # matmul moving free dim == one PSUM bank of fp32
NB = B // FD       # batch blocks
ICK = 4            # K-subtiles (x128 rows) per streamed input chunk
MCK = 4            # K-subtiles per weight/mask load+mask chunk

# Compute dtype: fp16 | bf16 | fp32r | fp32
DTYPE = os.environ.get("BASS_MLP_DTYPE", "fp16")

_cache = {}


def _np_cdt():
    if DTYPE == "bf16":
        import ml_dtypes

        return ml_dtypes.bfloat16
    return {"fp16": np.float16, "fp32r": np.float32, "fp32": np.float32}[DTYPE]


def _build():
    """Build + schedule the SPMD Bass program (same NEFF on all 8 cores)."""
    import concourse.tile as tile
    from concourse import bacc, mybir
    from concourse.bass import DynSlice

    cdt = {
        "fp16": mybir.dt.float16,
        "bf16": mybir.dt.bfloat16,
        "fp32r": mybir.dt.float32,   # storage dtype; matmul APs bitcast to f32r
        "fp32": mybir.dt.float32,
    }[DTYPE]
    mm_cast = mybir.dt.float32r if DTYPE == "fp32r" else None
    esz = mybir.dt.size(cdt)

    # Per-layer output-feature shard sizes and weight-panel widths.
    FS = [DIMS[1] // NCORES, DIMS[2] // NCORES, DIMS[3] // NCORES]  # 1024,1024,512
    KS = [DIMS[0], DIMS[1], DIMS[2]]
    if esz == 2:
        FBLK = [1024, 1024, 512]     # whole shard resident per layer
    else:
        FBLK = [1024, 512, 512]      # L2 split into two panels (SBUF)

    nc = bacc.Bacc(None, target_bir_lowering=False, debug=False, num_devices=NCORES)

    xT = nc.dram_tensor("xT", [KS[0], B], cdt, kind="ExternalInput")
    wts, mts, bs = [], [], []
    for li in range(3):
        wts.append(nc.dram_tensor(f"w{li + 1}t", [KS[li], FS[li]], cdt,
                                  kind="ExternalInput"))
        mts.append(nc.dram_tensor(f"m{li + 1}t", [KS[li], FS[li]], cdt,
                                  kind="ExternalInput"))
        bs.append(nc.dram_tensor(f"b{li + 1}", [FS[li]], mybir.dt.float32,
                                 kind="ExternalInput"))
    out = nc.dram_tensor("out", [FS[2], B], mybir.dt.float32,
                         kind="ExternalOutput")

    with tile.TileContext(nc) as tc:
        with tc.tile_pool(name="wp", bufs=1) as wpool, \
             tc.tile_pool(name="inp", bufs=6) as ipool, \
             tc.tile_pool(name="mp", bufs=2) as mpool, \
             tc.tile_pool(name="op", bufs=6) as opool, \
             tc.tile_pool(name="bp", bufs=3) as bpool, \
             tc.tile_pool(name="ps", bufs=8, space="PSUM") as pspool, \
             tc.tile_pool(name="dram", bufs=1, space="DRAM") as dram:

            h_loc = [dram.tile([FS[0], B], cdt, name="h1_loc"),
                     dram.tile([FS[1], B], cdt, name="h2_loc")]
            h_full = [dram.tile([DIMS[1], B], cdt, addr_space="Shared",
                                name="h1_full"),
                      dram.tile([DIMS[2], B], cdt, addr_space="Shared",
                                name="h2_full")]

            def layer(li, in_ap, tanh):
                K, F = KS[li], FS[li]
                KO = K // P
                wt_r = wts[li].ap().rearrange("(ko p) f -> p ko f", p=P)
                mt_r = mts[li].ap().rearrange("(ko p) f -> p ko f", p=P)
                in_r = in_ap.rearrange("(ko p) n -> p ko n", p=P)
                if li < 2:
                    dst = h_loc[li][:]
                else:
                    dst = out.ap()

                btile = bpool.tile([P, F // P], mybir.dt.float32, tag="bias",
                                   name=f"bias{li}")
                nc.sync.dma_start(btile[:], bs[li].ap().rearrange(
                    "(o p) -> p o", p=P))

                fblk = FBLK[li]
                for f0 in range(0, F, fblk):
                    # --- load + mask one weight panel [P, KO, fblk] ---
                    wp = wpool.tile([P, KO, fblk], cdt, tag="wpanel",
                                    name=f"wp{li}_{f0}")
                    for c0 in range(0, KO, MCK):
                        csl = slice(c0, c0 + MCK)
                        fsl = DynSlice(f0, fblk)
                        nc.sync.dma_start(wp[:, csl, :], wt_r[:, csl, fsl])
                        mtile = mpool.tile([P, MCK, fblk], cdt, tag="mchunk",
                                           name=f"m{li}_{f0}_{c0}")
                        nc.sync.dma_start(mtile[:], mt_r[:, csl, fsl])
                        nc.vector.tensor_tensor(wp[:, csl, :], wp[:, csl, :],
                                                mtile[:], mybir.AluOpType.mult)

                    nf = fblk // P
                    for b in range(NB):
                        bsl = DynSlice(b * FD, FD)
                        psums = [pspool.tile([P, FD], mybir.dt.float32,
                                             tag="ps", name=f"ps{li}_{f0}_{b}_{f}")
                                 for f in range(nf)]
                        for c0 in range(0, KO, ICK):
                            it = ipool.tile([P, ICK, FD], cdt, tag="instrip",
                                            name=f"in{li}_{f0}_{b}_{c0}")
                            nc.sync.dma_start(
                                it[:], in_r[:, slice(c0, c0 + ICK), bsl])
                            for f in range(nf):
                                for ks in range(ICK):
                                    ko = c0 + ks
                                    lhsT = wp[:, ko, DynSlice(f * P, P)]
                                    rhs = it[:, ks, :]
                                    if mm_cast is not None:
                                        lhsT = lhsT.bitcast(mm_cast)
                                        rhs = rhs.bitcast(mm_cast)
                                    nc.tensor.matmul(
                                        psums[f][:], lhsT, rhs,
                                        start=(ko == 0), stop=(ko == KO - 1))
                        for f in range(nf):
                            fg = f0 + f * P   # feature row offset in shard
                            odt = cdt if li < 2 else mybir.dt.float32
                            ot = opool.tile([P, FD], odt, tag="prod",
                                            name=f"o{li}_{f0}_{b}_{f}")
                            func = (mybir.ActivationFunctionType.Tanh if tanh
                                    else mybir.ActivationFunctionType.Identity)
                            nc.scalar.activation(
                                ot[:], psums[f][:], func,
                                bias=btile[:, DynSlice((f0 // P) + f, 1)])
                            nc.sync.dma_start(
                                dst[DynSlice(fg, P), bsl], ot[:])

                if li < 2:
                    nc.gpsimd.collective_compute(
                        "AllGather",
                        mybir.AluOpType.bypass,
                        replica_groups=[list(range(NCORES))],
                        ins=[h_loc[li].opt()],
                        outs=[h_full[li].opt()],
                    )

            layer(0, xT.ap(), tanh=True)
            layer(1, h_full[0][:], tanh=True)
            layer(2, h_full[1][:], tanh=False)

    nc.compile()
    return nc


def get_nc():
    if "nc" not in _cache:
        _cache["nc"] = _build()
    return _cache["nc"]


def make_in_maps(x, W1, b1, m1, W2, b2, m2, W3, b3, m3):
    """Host-side sharding: transpose to [K, F] layouts, cast, slice shards."""
    npdt = _np_cdt()
    xT = np.ascontiguousarray(x.T).astype(npdt, copy=False)
    Ws = [W1, W2, W3]
    Ms = [m1, m2, m3]
    Bs = [b1, b2, b3]
    in_maps = []
    for k in range(NCORES):
        m = {"xT": xT}
        for li in range(3):
            F = DIMS[li + 1]
            fs = F // NCORES
            sl = slice(k * fs, (k + 1) * fs)
            m[f"w{li + 1}t"] = np.ascontiguousarray(Ws[li][sl].T).astype(
                npdt, copy=False)
            m[f"m{li + 1}t"] = np.ascontiguousarray(Ms[li][sl].T).astype(npdt)
            m[f"b{li + 1}"] = np.ascontiguousarray(Bs[li][sl]).astype(
                np.float32, copy=False)
        in_maps.append(m)
    return in_maps


def kernel(x, W1, b1, m1, W2, b2, m2, W3, b3, m3):
    from concourse.bass_utils import run_bass_kernel_spmd

    nc = get_nc()
    in_maps = make_in_maps(x, W1, b1, m1, W2, b2, m2, W3, b3, m3)
    res = run_bass_kernel_spmd(nc, in_maps, core_ids=list(range(NCORES)))
    outT = np.concatenate([res.results[k]["out"] for k in range(NCORES)], axis=0)
    return np.ascontiguousarray(outT.T)
